# revision 5
# baseline (speedup 1.0000x reference)
"""DeepSimilarity forward kernel for 8 axon-tunneled trn2 NeuronCores.

Sharding: data-parallel over the batch axis (B=4). The two scrambled
row-major reshapes in the reference ((b,3,n,k)->(b*n,3,k) and
(b,7,n,k)->(b,7k,n)) couple the entire n-axis of each batch element, so
batch is the only cleanly independent axis; each batch element's full
forward runs on its own NeuronCore (4 cores busy, dispatched
asynchronously so they run concurrently).

Precision: --auto-cast=none keeps all matmuls fp32. The ball-query
compare (d2 < r^2) flips membership for ~1e-6-near-boundary pairs if
matmuls are downcast, which corrupts the gathered neighborhoods.
"""

import os

os.environ.setdefault("NEURON_CC_FLAGS", "--auto-cast=none")

import numpy as np
import jax
import jax.numpy as jnp

RADIUS = 0.08
KMAX = 64
B, N, M = 4, 4096, 4096


def _ball_query(pts, ref, radius, k):
    # pts (1,3,n), ref (1,3,m) -> (1,n,k) int32, PointNet++ first-k-in-order
    m = ref.shape[2]
    d2 = (jnp.sum(pts * pts, 1)[:, :, None] + jnp.sum(ref * ref, 1)[:, None, :]
          - 2.0 * jnp.einsum('bcn,bcm->bnm', pts, ref))
    mask = d2 < radius * radius
    # neuron TopK rejects int32 inputs; keys < 2^24 are exact in f32
    ar = jnp.arange(m, dtype=jnp.float32)
    key = jnp.where(mask, ar, ar + m)
    neg_top = jax.lax.top_k(-key, k)[0]
    idx = (-neg_top).astype(jnp.int32) % m
    cnt = jnp.sum(mask, axis=-1)
    valid = jnp.arange(k, dtype=jnp.int32)[None, None, :] < cnt[:, :, None]
    return jnp.where(valid, idx, idx[..., :1])


def _gather(ref, idx):
    # ref (1,3,m), idx (1,n,k) -> (1,3,n,k)
    # chunk along n: one big indirect load overflows neuronxcc's 16-bit
    # semaphore_wait_value field (NCC_IXCG967)
    n = idx.shape[1]
    step = 512
    outs = [jax.vmap(lambda r, i: r[:, i])(ref, idx[:, s:s + step])
            for s in range(0, n, step)]
    return jnp.concatenate(outs, axis=2)


def _fwd(pts, knn, img, mlp1, mlp2, mlp3, conv):
    """Full forward for one batch element. pts/knn/img: (1,3,n)."""
    i_r1 = _ball_query(pts, knn, RADIUS, KMAX)
    i_r2 = _ball_query(pts, knn, RADIUS / 2, KMAX // 2)
    i_r3 = _ball_query(pts, knn, RADIUS / 4, KMAX // 4)
    i_i1 = _ball_query(pts, img, RADIUS, KMAX)

    def collect(idxr, k):
        realn = _gather(knn, idxr)                       # (1,3,n,k)
        diff = realn - pts[:, :, :, None]
        dist = jnp.sum(diff * diff, axis=1, keepdims=True)
        b = realn.shape[0]
        n = realn.shape[2]
        # mirrors torch's row-major reshape (b,7,n,k) -> (b,7k,n)
        rep = jnp.concatenate([realn, diff, dist], 1).reshape(b, 7 * k, n)
        return realn, rep

    realn1, rep1 = collect(i_r1, KMAX)
    _, rep2 = collect(i_r2, KMAX // 2)
    _, rep3 = collect(i_r3, KMAX // 4)
    imgn1 = _gather(img, i_i1)

    b, _, n, k = realn1.shape
    # mirrors torch's row-major reshape (b,3,n,k) -> (b*n,3,k)
    a = realn1.reshape(b * n, 3, k)
    bb = imgn1.reshape(b * n, 3, k)
    d2p = (jnp.sum(a * a, 1)[:, :, None] + jnp.sum(bb * bb, 1)[:, None, :]
           - 2.0 * jnp.einsum('nck,ncl->nkl', a, bb))
    mind = jnp.min(d2p, axis=-1)                          # (b*n, k)
    ls1 = (1.0 - jax.nn.sigmoid(jnp.mean(mind, axis=-1))).reshape(b, n)

    def mlp(x, layers):
        for W, bv in layers:
            x = jax.nn.relu(jnp.einsum('bcn,cd->bdn', x, W) + bv[None, :, None])
        return x

    f1 = mlp(rep1, mlp1)
    f2 = mlp(rep2, mlp2)
    f3 = mlp(rep3, mlp3)
    feat = jnp.concatenate([f1, f2, f3], axis=1)          # (1,160,n)
    W, bv = conv
    out = jnp.einsum('bcn,cd->bdn', feat, W) + bv[None, :, None]
    return ls1 * 2.0, out


_cached = {}


def _get_pmapped():
    # pmap: one executable shared by the 4 devices (a per-device jit would
    # recompile the ~4-minute module once per core)
    if 'fn' not in _cached:
        _cached['fn'] = jax.pmap(
            _fwd, devices=jax.devices()[:B],
            in_axes=(0, 0, 0, None, None, None, None))
    return _cached['fn']


def kernel(points, knnpoints, imageneighbors, is_training, params):
    pts = np.asarray(points, dtype=np.float32)
    knn = np.asarray(knnpoints, dtype=np.float32)
    img = np.asarray(imageneighbors, dtype=np.float32)

    def tolayers(ls):
        return tuple((jnp.asarray(W, jnp.float32), jnp.asarray(bv, jnp.float32))
                     for W, bv in ls)

    mlp1 = tolayers(params['mlp1'])
    mlp2 = tolayers(params['mlp2'])
    mlp3 = tolayers(params['mlp3'])
    convW, convb = params['conv']
    conv = (jnp.asarray(convW, jnp.float32), jnp.asarray(convb, jnp.float32))

    fn = _get_pmapped()
    # batch element b -> NeuronCore b; weights replicated
    ss, feat = fn(pts[:, None], knn[:, None], img[:, None],
                  mlp1, mlp2, mlp3, conv)
    ss = np.asarray(ss)[:, 0]        # (4, n)
    feat = np.asarray(feat)[:, 0]    # (4, 64, n)

    shallow_similarity = ss.astype(np.float32)
    deep_similarity = np.ones_like(shallow_similarity)
    shallowrealfeat = feat.astype(np.float32)
    return shallow_similarity, deep_similarity, shallowrealfeat
